# revision 57
# baseline (speedup 1.0000x reference)
import sys

sys.path.insert(0, "/opt/trn_rl_repo")

import numpy as np
import ml_dtypes

import concourse.bass as bass
import concourse.tile as tile
from concourse import bacc, mybir
from concourse.bass_utils import run_bass_kernel_spmd

# ---- problem constants (hardcoded per contract) ----
B, N, F = 8, 512, 16
D, PP = 150, 26
IMG = 128
NB = 4                  # render batches of 128 z-sorted emitters
NCHUNK = 16             # 128-row K chunks per batch (2 z-slabs of 64);
                        # bin-packed batches have <=30 distinct z each
RTB = 8                 # rt chunks per DMA
RN = PP * PP            # 676
W184 = 160              # dram row stride (elements); zero gap 134 >= 113 max backread
LEAD = 8                # leading zero rows per partition
TAIL = 7                # trailing zero rows
JROWS = LEAD + NB * PP + TAIL   # 119
SEG = JROWS * W184      # 21896 elements per partition
NBLK = 16
NSP = 4                 # spill cells, each serving 4 blocks
QI = NBLK + NSP         # 16 main octs + 4 spill quads = 20 gather chunks
CHO = 7 * W184 + 128    # 1416 elements per oct chunk (8 rows)
CHS = 3 * W184 + 128    # 680 elements per spill chunk (4 rows)
XW = 128                # canvas x window [26,154)

_compiled = None


def _build_bass(debug=False):
    nc = bacc.Bacc()
    f32 = mybir.dt.float32
    bf16 = mybir.dt.bfloat16
    i32 = mybir.dt.int32

    rlhs_d = nc.declare_dram_parameter(
        "rlhs", [NB, 128, NCHUNK * 128], bf16, isOutput=False
    )
    rslab_d = nc.declare_dram_parameter(
        "rslab", [NB, NCHUNK // RTB, 128, RTB * RN], bf16, isOutput=False
    )
    idx_d = nc.declare_dram_parameter("idx", [128, QI], i32, isOutput=False)
    # main oct rows compact (cols (row%8)*16..+16 only); spill rows full width
    rowc_d = nc.declare_dram_parameter("rowc", [128, 128 * 16], bf16, isOutput=False)
    rowlhs_d = nc.declare_dram_parameter(
        "rowlhs", [128, 64 * 128], bf16, isOutput=False
    )
    out_d = nc.declare_dram_parameter("out", [128, NBLK * XW], bf16, isOutput=True)
    dbg = {}
    if debug:
        dbg["rend"] = nc.declare_dram_parameter(
            "dbg_rend", [128, SEG], bf16, isOutput=True
        )

    with tile.TileContext(nc) as tc:
        with (
            tc.tile_pool(name="big", bufs=1) as big_pool,
            tc.tile_pool(name="lt", bufs=2) as lt_pool,
            tc.tile_pool(name="rt", bufs=4) as rt_pool,
            tc.tile_pool(name="psr", bufs=2, space="PSUM") as psr_pool,
            tc.tile_pool(name="psc", bufs=4, space="PSUM") as psc_pool,
            tc.tile_pool(name="dram", bufs=1, space="DRAM") as dram_pool,
        ):
            idx_t = big_pool.tile([128, QI], i32, tag="idx")

            rend = big_pool.tile([128, SEG], bf16, tag="rend")
            nc.vector.memset(rend[:].bitcast(i32), 0)

            rows_d = dram_pool.tile([128, SEG], bf16, tag="rows")
            rowlhs_t = big_pool.tile([128, 192 * 128], bf16, tag="rowlhs")
            rowc_t = big_pool.tile([128, 128 * 16], bf16, tag="rowc")
            # zero the main-row region; DVE copies later fill the 16-col blocks
            nc.vector.memset(rowlhs_t[:, : 128 * 128].bitcast(i32), 0)

            # zero lead + tail rows depend only on the memset; dump them early.
            # dumps ride the (otherwise idle) gpsimd SWDGE ring so the sync
            # ring stays clear for the rt slab stream.
            nc.gpsimd.dma_start(rows_d[:, 0 : LEAD * W184], rend[:, 0 : LEAD * W184])
            nc.gpsimd.dma_start(
                rows_d[:, (JROWS - TAIL) * W184 :], rend[:, (JROWS - TAIL) * W184 :]
            )

            for m in range(NB):
                lt = lt_pool.tile([128, NCHUNK * 128], bf16, tag="lt")
                nc.sync.dma_start(lt[:], rlhs_d[m])
                ps = psr_pool.tile([128, RN], f32, tag="ps")
                for cb in range(NCHUNK // RTB):
                    rt = rt_pool.tile([128, RTB * RN], bf16, tag="rt")
                    (nc.scalar if cb % 2 == 0 else nc.sync).dma_start(
                        rt[:], rslab_d[m, cb]
                    )
                    for cc in range(RTB):
                        c = cb * RTB + cc
                        for n0, n1 in ((0, 512), (512, RN)):
                            nc.tensor.matmul(
                                ps[:, n0:n1],
                                lhsT=lt[:, c * 128 : (c + 1) * 128],
                                rhs=rt[:, cc * RN + n0 : cc * RN + n1],
                                start=(c == 0),
                                stop=(c == NCHUNK - 1),
                            )
                if m == 2:
                    # idx only needed by the gathers; load out of startup path
                    nc.scalar.dma_start(idx_t[:], idx_d[:])
                if m == 3:
                    # row-MM tables load late (render bandwidth) but small
                    # enough (2.9MB) to complete alongside the last dump
                    nc.scalar.dma_start(rowc_t[:], rowc_d[:])
                    nc.scalar.dma_start(
                        rowlhs_t[:, 128 * 128 :], rowlhs_d[:]
                    )
                    # expand compact main rows on the (idle) DVE:
                    # row blk*8+k only has cols [k*16, k*16+16) nonzero
                    rl4 = (
                        rowlhs_t[:]
                        .rearrange("p (r c) -> p r c", r=192)[:, 0:128, :]
                        .rearrange("p (b k) c -> p b k c", k=8)
                    )
                    rc4 = rowc_t[:].rearrange("p (b k f) -> p b k f", k=8, f=16)
                    for k in range(8):
                        nc.vector.tensor_copy(
                            rl4[:, :, k, k * 16 : (k + 1) * 16], rc4[:, :, k, :]
                        )
                # copy+dump in halves so the final dump completes sooner; the
                # copies run on the (render-idle) DVE so the scalar sequencer
                # stays free for its slab DMA issues
                for r0, r1 in ((0, 13), (13, 26)):
                    nc.vector.tensor_copy(
                        rend[:].rearrange("p (j w) -> p j w", j=JROWS)[
                            :, m * 26 + LEAD + r0 : m * 26 + LEAD + r1, 0:26
                        ],
                        ps[:].rearrange("p (h w) -> p h w", h=26)[:, r0:r1, :],
                    )
                    nc.gpsimd.dma_start(
                        rows_d[
                            :,
                            (m * 26 + LEAD + r0) * W184 : (m * 26 + LEAD + r1) * W184,
                        ],
                        rend[
                            :,
                            (m * 26 + LEAD + r0) * W184 : (m * 26 + LEAD + r1) * W184,
                        ],
                    )
            strip_o = big_pool.tile([128, NBLK * CHO], bf16, tag="strip_o")
            strip_o3 = strip_o[:].rearrange("p (q x) -> p q x", q=NBLK)
            strip_s = big_pool.tile([128, NSP * CHS], bf16, tag="strip_s")
            strip_s3 = strip_s[:].rearrange("p (q x) -> p q x", q=NSP)
            rows_flat = rows_d[:].rearrange("p (n o) -> (p n) o", o=1)
            # first oct, then the group's spill cell, then remaining octs:
            # each block's main row-MMs start at its own oct, spill data is
            # already there, and the tail waits only on the last single oct
            for j in range(NSP):
                for pos, col in enumerate(
                    [4 * j, NBLK + j, 4 * j + 1, 4 * j + 2, 4 * j + 3]
                ):
                    out_ap = (
                        strip_s3[:, j, :] if pos == 1 else strip_o3[:, col, :]
                    )
                    nc.gpsimd.indirect_dma_start(
                        out=out_ap,
                        out_offset=None,
                        in_=rows_flat,
                        in_offset=bass.IndirectOffsetOnAxis(
                            ap=idx_t[:, col : col + 1], axis=0
                        ),
                    )

            if debug:
                nc.sync.dma_start(dbg["rend"][:], rend[:])

            canvas = big_pool.tile([128, NBLK * XW], bf16, tag="canvas")
            rowlhs3 = rowlhs_t[:].rearrange("p (y m) -> p y m", y=192)
            for blk in range(NBLK):
                pc = psc_pool.tile([128, XW], f32, tag="pc")
                for k in range(8):
                    nc.tensor.matmul(
                        pc[:],
                        lhsT=rowlhs3[:, blk * 8 + k, :],
                        rhs=strip_o3[:, blk, k * W184 : k * W184 + XW],
                        start=(k == 0),
                        stop=False,
                    )
                for k in range(4):
                    nc.tensor.matmul(
                        pc[:],
                        lhsT=rowlhs3[:, 128 + blk * 4 + k, :],
                        rhs=strip_s3[:, blk // 4, k * W184 : k * W184 + XW],
                        start=False,
                        stop=(k == 3),
                    )
                nc.scalar.copy(
                    out=canvas[:, blk * XW : (blk + 1) * XW], in_=pc[:]
                )
                # ship each block as soon as it drains (tail shrinks to the
                # last block's small DMA)
                (nc.sync if blk % 2 == 0 else nc.scalar).dma_start(
                    out_d[:, blk * XW : (blk + 1) * XW],
                    canvas[:, blk * XW : (blk + 1) * XW],
                )
    if not nc.is_finalized():
        nc.finalize()
    return nc


def _host_prep(xyz, n_photons, coeffs, inv_voxel_size, psf_center):
    u = xyz * inv_voxel_size
    u = u.copy()
    u[..., :2] -= psf_center[:2]
    u[..., 2] += psf_center[2]
    u_floor = np.floor(u)
    frac = u - u_floor
    ui = u_floor.astype(np.int32)
    x_idx = ui[..., 0] + PP
    y_idx = ui[..., 1] + PP
    z_idx = ui[..., 2]
    frac[..., :2] = 1.0 - frac[..., :2]

    p4 = frac[..., None] ** np.arange(4, dtype=np.float32)
    vx, vy, vz = p4[..., 0, :], p4[..., 1, :], p4[..., 2, :]
    series = (
        vz[..., :, None, None] * vx[..., None, :, None] * vy[..., None, None, :]
    ).reshape(B, N, 64)

    slab = np.ascontiguousarray(coeffs.transpose(0, 3, 1, 2).reshape(D, 64, RN))

    in_maps = []
    for b in range(B):
        in_maps.append(
            _prep_one(x_idx[b], y_idx[b], z_idx[b], series[b], n_photons[b], slab)
        )
    return in_maps


def _prep_one(x_idx, y_idx, z_idx, series, photons, slab):
    # bin-pack z-groups into NB batches of exactly 128 emitters, balancing
    # distinct-z per batch (<= 2*NCHUNK); batches need not be z-contiguous
    zs = np.unique(z_idx)
    groups = [np.where(z_idx == z)[0] for z in zs]
    groups.sort(key=len, reverse=True)
    counts = [0] * NB
    nz = [0] * NB
    bins = [[] for _ in range(NB)]
    for g in groups:
        take = list(g)
        while take:
            cand = [i for i in range(NB) if counts[i] < 128]
            i = min(cand, key=lambda i: (counts[i], nz[i]))
            space = 128 - counts[i]
            put = take[:space]
            bins[i].extend(put)
            counts[i] += len(put)
            nz[i] += 1
            take = take[space:]
    assert all(c == 128 for c in counts)
    assert max(nz) <= 2 * NCHUNK, nz
    order = np.array([e for b in bins for e in b], dtype=np.int64)
    pos = np.empty(N, dtype=np.int64)
    pos[order] = np.arange(N)

    rlhs = np.zeros((NB, NCHUNK, 128, 128), dtype=np.float32)
    rslab = np.zeros((NB, NCHUNK, 128, RN), dtype=ml_dtypes.bfloat16)
    for m in range(NB):
        es = order[m * 128 : (m + 1) * 128]
        zlist = np.unique(z_idx[es])
        assert len(zlist) <= 2 * NCHUNK
        zpos = {z: i for i, z in enumerate(zlist)}
        for i, z in enumerate(zlist):
            rslab[m, i // 2, 64 * (i % 2) : 64 * (i % 2) + 64, :] = slab[z]
        for col, e in enumerate(es):
            zi = zpos[z_idx[e]]
            rlhs[m, zi // 2, 64 * (zi % 2) : 64 * (zi % 2) + 64, col] = series[e]

    # per-block packing: main cells are 8-row octs (idx col = blk, lhsT rows
    # blk*8+k), overflow contributors split into 4-row spill windows (idx col
    # = NBLK+blk, lhsT rows 128 + blk*4+k); junk rows get zero photon weights
    idx = np.full((128, QI), 26, dtype=np.int32)
    rowlhs = np.zeros((128, 192, 128), dtype=np.float32)
    fill = np.zeros(QI, dtype=np.int64)

    def emit(cell, y0, nrows, lrow0, e):
        """emitter e's piece with an nrows-row window starting at canvas-crop
        row y0, placed in gather column `cell`, lhsT rows lrow0+k"""
        s = fill[cell]
        assert s < 128, f"cell overflow {cell}"
        fill[cell] += 1
        r0 = y0 + 26 - y_idx[e]  # patch row at window row 0 (may be <0)
        p = pos[e] % 128
        jb = (pos[e] // 128) * 26 + LEAD
        idx[s, cell] = p * SEG + (jb + r0) * W184 + 26 - x_idx[e]
        for k in range(nrows):
            Y = y0 + k
            r = r0 + k
            if 0 <= r < 26 and 0 <= Y < IMG:
                col0 = (Y % 8) * 16
                rowlhs[s, lrow0 + k, col0 : col0 + 16] = photons[e]

    for blk in range(NBLK):
        es = np.where((y_idx > 8 * blk) & (y_idx <= 8 * blk + 33))[0]
        for i, e in enumerate(es):
            if i < 128:
                emit(blk, 8 * blk, 8, blk * 8, e)
            else:
                for h in range(2):
                    y0 = 8 * blk + 4 * h
                    if (y_idx[e] > y0) and (y_idx[e] <= y0 + 29):
                        emit(NBLK + blk // 4, y0, 4, 128 + blk * 4, e)

    # partition-contiguous DRAM layouts (128 descriptors per DMA)
    rlhs_t = np.ascontiguousarray(rlhs.transpose(0, 2, 1, 3)).reshape(
        NB, 128, NCHUNK * 128
    )
    rslab_t = np.ascontiguousarray(
        rslab.reshape(NB, NCHUNK // RTB, RTB, 128, RN).transpose(0, 1, 3, 2, 4)
    ).reshape(NB, NCHUNK // RTB, 128, RTB * RN)
    # compact main rows: row blk*8+k only has cols [(k)*16, (k+1)*16) nonzero
    rowc = np.zeros((128, 128, 16), dtype=np.float32)
    for k in range(8):
        rowc[:, k::8, :] = rowlhs[:, 0:128, :][:, k::8, k * 16 : (k + 1) * 16]
    return {
        "rlhs": rlhs_t.astype(ml_dtypes.bfloat16),
        "rslab": rslab_t,
        "idx": idx,
        "rowc": rowc.astype(ml_dtypes.bfloat16).reshape(128, 128 * 16),
        "rowlhs": rowlhs[:, 128:, :].astype(ml_dtypes.bfloat16).reshape(
            128, 64 * 128
        ),
    }


def kernel(xyz, n_photons, coeffs, inv_voxel_size, psf_center, img_size):
    global _compiled
    xyz = np.asarray(xyz, dtype=np.float32)
    n_photons = np.asarray(n_photons, dtype=np.float32)
    coeffs = np.asarray(coeffs, dtype=np.float32)
    inv_voxel_size = np.asarray(inv_voxel_size, dtype=np.float32)
    psf_center = np.asarray(psf_center, dtype=np.float32)

    in_maps = _host_prep(xyz, n_photons, coeffs, inv_voxel_size, psf_center)

    if _compiled is None:
        _compiled = _build_bass()
    nc = _compiled

    res = run_bass_kernel_spmd(nc, in_maps, core_ids=list(range(B)))
    outs = []
    for b in range(B):
        c4 = res.results[b]["out"].astype(np.float32).reshape(8, 16, NBLK, XW)
        outs.append(c4.transpose(1, 2, 0, 3).reshape(F, IMG, IMG))
    return np.stack(outs, axis=0)


# revision 58
# speedup vs baseline: 1.0608x; 1.0608x over previous
import sys

sys.path.insert(0, "/opt/trn_rl_repo")

import numpy as np
import ml_dtypes

import concourse.bass as bass
import concourse.tile as tile
from concourse import bacc, mybir
from concourse.bass_utils import run_bass_kernel_spmd

# ---- problem constants (hardcoded per contract) ----
B, N, F = 8, 512, 16
D, PP = 150, 26
IMG = 128
NB = 4                  # render batches of 128 z-sorted emitters
NCHUNK = 16             # 128-row K chunks per batch (2 z-slabs of 64);
                        # bin-packed batches have <=30 distinct z each
RTB = 8                 # rt chunks per DMA
RN = PP * PP            # 676
W184 = 160              # dram row stride (elements); zero gap 134 >= 113 max backread
LEAD = 8                # leading zero rows per partition
TAIL = 7                # trailing zero rows
JROWS = LEAD + NB * PP + TAIL   # 119
SEG = JROWS * W184      # 21896 elements per partition
NBLK = 16
NSP = 4                 # spill cells, each serving 4 blocks
QI = NBLK + NSP         # 16 main octs + 4 spill quads = 20 gather chunks
CHO = 7 * W184 + 128    # 1416 elements per oct chunk (8 rows)
CHS = 3 * W184 + 128    # 680 elements per spill chunk (4 rows)
XW = 128                # canvas x window [26,154)

_compiled = None


def _build_bass(debug=False):
    nc = bacc.Bacc()
    f32 = mybir.dt.float32
    bf16 = mybir.dt.bfloat16
    i32 = mybir.dt.int32

    rlhs_d = nc.declare_dram_parameter(
        "rlhs", [NB, 128, NCHUNK * 128], bf16, isOutput=False
    )
    rslab_d = nc.declare_dram_parameter(
        "rslab", [NB, NCHUNK // RTB, 128, RTB * RN], bf16, isOutput=False
    )
    idx_d = nc.declare_dram_parameter("idx", [128, QI], i32, isOutput=False)
    # main oct rows compact (cols (row%8)*16..+16 only); spill rows full width
    rowc_d = nc.declare_dram_parameter("rowc", [128, 128 * 16], bf16, isOutput=False)
    rowlhs_d = nc.declare_dram_parameter(
        "rowlhs", [128, 64 * 128], bf16, isOutput=False
    )
    out_d = nc.declare_dram_parameter("out", [128, NBLK * XW], bf16, isOutput=True)
    dbg = {}
    if debug:
        dbg["rend"] = nc.declare_dram_parameter(
            "dbg_rend", [128, SEG], bf16, isOutput=True
        )

    with tile.TileContext(nc) as tc:
        with (
            tc.tile_pool(name="big", bufs=1) as big_pool,
            tc.tile_pool(name="lt", bufs=2) as lt_pool,
            tc.tile_pool(name="rt", bufs=5) as rt_pool,
            tc.tile_pool(name="psr", bufs=2, space="PSUM") as psr_pool,
            tc.tile_pool(name="psc", bufs=4, space="PSUM") as psc_pool,
            tc.tile_pool(name="dram", bufs=1, space="DRAM") as dram_pool,
        ):
            idx_t = big_pool.tile([128, QI], i32, tag="idx")

            rend = big_pool.tile([128, SEG], bf16, tag="rend")
            nc.vector.memset(rend[:].bitcast(i32), 0)

            rows_d = dram_pool.tile([128, SEG], bf16, tag="rows")
            rowlhs_t = big_pool.tile([128, 192 * 128], bf16, tag="rowlhs")
            rowc_t = big_pool.tile([128, 128 * 16], bf16, tag="rowc")
            # zero the main-row region; DVE copies later fill the 16-col blocks
            nc.vector.memset(rowlhs_t[:, : 128 * 128].bitcast(i32), 0)

            # zero lead + tail rows depend only on the memset; dump them early.
            # dumps ride the (otherwise idle) gpsimd SWDGE ring so the sync
            # ring stays clear for the rt slab stream.
            nc.gpsimd.dma_start(rows_d[:, 0 : LEAD * W184], rend[:, 0 : LEAD * W184])
            nc.gpsimd.dma_start(
                rows_d[:, (JROWS - TAIL) * W184 :], rend[:, (JROWS - TAIL) * W184 :]
            )

            for m in range(NB):
                lt = lt_pool.tile([128, NCHUNK * 128], bf16, tag="lt")
                nc.sync.dma_start(lt[:], rlhs_d[m])
                ps = psr_pool.tile([128, RN], f32, tag="ps")
                for cb in range(NCHUNK // RTB):
                    rt = rt_pool.tile([128, RTB * RN], bf16, tag="rt")
                    (nc.scalar if cb % 2 == 0 else nc.sync).dma_start(
                        rt[:], rslab_d[m, cb]
                    )
                    for cc in range(RTB):
                        c = cb * RTB + cc
                        for n0, n1 in ((0, 512), (512, RN)):
                            nc.tensor.matmul(
                                ps[:, n0:n1],
                                lhsT=lt[:, c * 128 : (c + 1) * 128],
                                rhs=rt[:, cc * RN + n0 : cc * RN + n1],
                                start=(c == 0),
                                stop=(c == NCHUNK - 1),
                            )
                if m == 2:
                    # idx only needed by the gathers; load out of startup path
                    nc.scalar.dma_start(idx_t[:], idx_d[:])
                if m == 3:
                    # row-MM tables load late (render bandwidth) but small
                    # enough (2.9MB) to complete alongside the last dump
                    nc.scalar.dma_start(rowc_t[:], rowc_d[:])
                    nc.scalar.dma_start(
                        rowlhs_t[:, 128 * 128 :], rowlhs_d[:]
                    )
                    # expand compact main rows on the (idle) DVE:
                    # row blk*8+k only has cols [k*16, k*16+16) nonzero
                    rl4 = (
                        rowlhs_t[:]
                        .rearrange("p (r c) -> p r c", r=192)[:, 0:128, :]
                        .rearrange("p (b k) c -> p b k c", k=8)
                    )
                    rc4 = rowc_t[:].rearrange("p (b k f) -> p b k f", k=8, f=16)
                    for k in range(8):
                        nc.vector.tensor_copy(
                            rl4[:, :, k, k * 16 : (k + 1) * 16], rc4[:, :, k, :]
                        )
                # copy+dump in halves so the final dump completes sooner; the
                # copies run on the (render-idle) DVE so the scalar sequencer
                # stays free for its slab DMA issues
                for r0, r1 in ((0, 13), (13, 26)):
                    nc.vector.tensor_copy(
                        rend[:].rearrange("p (j w) -> p j w", j=JROWS)[
                            :, m * 26 + LEAD + r0 : m * 26 + LEAD + r1, 0:26
                        ],
                        ps[:].rearrange("p (h w) -> p h w", h=26)[:, r0:r1, :],
                    )
                    nc.gpsimd.dma_start(
                        rows_d[
                            :,
                            (m * 26 + LEAD + r0) * W184 : (m * 26 + LEAD + r1) * W184,
                        ],
                        rend[
                            :,
                            (m * 26 + LEAD + r0) * W184 : (m * 26 + LEAD + r1) * W184,
                        ],
                    )
            strip_o = big_pool.tile([128, NBLK * CHO], bf16, tag="strip_o")
            strip_o3 = strip_o[:].rearrange("p (q x) -> p q x", q=NBLK)
            strip_s = big_pool.tile([128, NSP * CHS], bf16, tag="strip_s")
            strip_s3 = strip_s[:].rearrange("p (q x) -> p q x", q=NSP)
            rows_flat = rows_d[:].rearrange("p (n o) -> (p n) o", o=1)
            # first oct, then the group's spill cell, then remaining octs:
            # each block's main row-MMs start at its own oct, spill data is
            # already there, and the tail waits only on the last single oct
            for j in range(NSP):
                for pos, col in enumerate(
                    [4 * j, NBLK + j, 4 * j + 1, 4 * j + 2, 4 * j + 3]
                ):
                    out_ap = (
                        strip_s3[:, j, :] if pos == 1 else strip_o3[:, col, :]
                    )
                    nc.gpsimd.indirect_dma_start(
                        out=out_ap,
                        out_offset=None,
                        in_=rows_flat,
                        in_offset=bass.IndirectOffsetOnAxis(
                            ap=idx_t[:, col : col + 1], axis=0
                        ),
                    )

            if debug:
                nc.sync.dma_start(dbg["rend"][:], rend[:])

            canvas = big_pool.tile([128, NBLK * XW], bf16, tag="canvas")
            rowlhs3 = rowlhs_t[:].rearrange("p (y m) -> p y m", y=192)
            for blk in range(NBLK):
                pc = psc_pool.tile([128, XW], f32, tag="pc")
                for k in range(8):
                    nc.tensor.matmul(
                        pc[:],
                        lhsT=rowlhs3[:, blk * 8 + k, :],
                        rhs=strip_o3[:, blk, k * W184 : k * W184 + XW],
                        start=(k == 0),
                        stop=False,
                    )
                for k in range(4):
                    nc.tensor.matmul(
                        pc[:],
                        lhsT=rowlhs3[:, 128 + blk * 4 + k, :],
                        rhs=strip_s3[:, blk // 4, k * W184 : k * W184 + XW],
                        start=False,
                        stop=(k == 3),
                    )
                nc.scalar.copy(
                    out=canvas[:, blk * XW : (blk + 1) * XW], in_=pc[:]
                )
                # ship each block as soon as it drains (tail shrinks to the
                # last block's small DMA)
                (nc.sync if blk % 2 == 0 else nc.scalar).dma_start(
                    out_d[:, blk * XW : (blk + 1) * XW],
                    canvas[:, blk * XW : (blk + 1) * XW],
                )
    if not nc.is_finalized():
        nc.finalize()
    return nc


def _host_prep(xyz, n_photons, coeffs, inv_voxel_size, psf_center):
    u = xyz * inv_voxel_size
    u = u.copy()
    u[..., :2] -= psf_center[:2]
    u[..., 2] += psf_center[2]
    u_floor = np.floor(u)
    frac = u - u_floor
    ui = u_floor.astype(np.int32)
    x_idx = ui[..., 0] + PP
    y_idx = ui[..., 1] + PP
    z_idx = ui[..., 2]
    frac[..., :2] = 1.0 - frac[..., :2]

    p4 = frac[..., None] ** np.arange(4, dtype=np.float32)
    vx, vy, vz = p4[..., 0, :], p4[..., 1, :], p4[..., 2, :]
    series = (
        vz[..., :, None, None] * vx[..., None, :, None] * vy[..., None, None, :]
    ).reshape(B, N, 64)

    slab = np.ascontiguousarray(coeffs.transpose(0, 3, 1, 2).reshape(D, 64, RN))

    in_maps = []
    for b in range(B):
        in_maps.append(
            _prep_one(x_idx[b], y_idx[b], z_idx[b], series[b], n_photons[b], slab)
        )
    return in_maps


def _prep_one(x_idx, y_idx, z_idx, series, photons, slab):
    # bin-pack z-groups into NB batches of exactly 128 emitters, balancing
    # distinct-z per batch (<= 2*NCHUNK); batches need not be z-contiguous
    zs = np.unique(z_idx)
    groups = [np.where(z_idx == z)[0] for z in zs]
    groups.sort(key=len, reverse=True)
    counts = [0] * NB
    nz = [0] * NB
    bins = [[] for _ in range(NB)]
    for g in groups:
        take = list(g)
        while take:
            cand = [i for i in range(NB) if counts[i] < 128]
            i = min(cand, key=lambda i: (counts[i], nz[i]))
            space = 128 - counts[i]
            put = take[:space]
            bins[i].extend(put)
            counts[i] += len(put)
            nz[i] += 1
            take = take[space:]
    assert all(c == 128 for c in counts)
    assert max(nz) <= 2 * NCHUNK, nz
    order = np.array([e for b in bins for e in b], dtype=np.int64)
    pos = np.empty(N, dtype=np.int64)
    pos[order] = np.arange(N)

    rlhs = np.zeros((NB, NCHUNK, 128, 128), dtype=np.float32)
    rslab = np.zeros((NB, NCHUNK, 128, RN), dtype=ml_dtypes.bfloat16)
    for m in range(NB):
        es = order[m * 128 : (m + 1) * 128]
        zlist = np.unique(z_idx[es])
        assert len(zlist) <= 2 * NCHUNK
        zpos = {z: i for i, z in enumerate(zlist)}
        for i, z in enumerate(zlist):
            rslab[m, i // 2, 64 * (i % 2) : 64 * (i % 2) + 64, :] = slab[z]
        for col, e in enumerate(es):
            zi = zpos[z_idx[e]]
            rlhs[m, zi // 2, 64 * (zi % 2) : 64 * (zi % 2) + 64, col] = series[e]

    # per-block packing: main cells are 8-row octs (idx col = blk, lhsT rows
    # blk*8+k), overflow contributors split into 4-row spill windows (idx col
    # = NBLK+blk, lhsT rows 128 + blk*4+k); junk rows get zero photon weights
    idx = np.full((128, QI), 26, dtype=np.int32)
    rowlhs = np.zeros((128, 192, 128), dtype=np.float32)
    fill = np.zeros(QI, dtype=np.int64)

    def emit(cell, y0, nrows, lrow0, e):
        """emitter e's piece with an nrows-row window starting at canvas-crop
        row y0, placed in gather column `cell`, lhsT rows lrow0+k"""
        s = fill[cell]
        assert s < 128, f"cell overflow {cell}"
        fill[cell] += 1
        r0 = y0 + 26 - y_idx[e]  # patch row at window row 0 (may be <0)
        p = pos[e] % 128
        jb = (pos[e] // 128) * 26 + LEAD
        idx[s, cell] = p * SEG + (jb + r0) * W184 + 26 - x_idx[e]
        for k in range(nrows):
            Y = y0 + k
            r = r0 + k
            if 0 <= r < 26 and 0 <= Y < IMG:
                col0 = (Y % 8) * 16
                rowlhs[s, lrow0 + k, col0 : col0 + 16] = photons[e]

    for blk in range(NBLK):
        es = np.where((y_idx > 8 * blk) & (y_idx <= 8 * blk + 33))[0]
        for i, e in enumerate(es):
            if i < 128:
                emit(blk, 8 * blk, 8, blk * 8, e)
            else:
                for h in range(2):
                    y0 = 8 * blk + 4 * h
                    if (y_idx[e] > y0) and (y_idx[e] <= y0 + 29):
                        emit(NBLK + blk // 4, y0, 4, 128 + blk * 4, e)

    # partition-contiguous DRAM layouts (128 descriptors per DMA)
    rlhs_t = np.ascontiguousarray(rlhs.transpose(0, 2, 1, 3)).reshape(
        NB, 128, NCHUNK * 128
    )
    rslab_t = np.ascontiguousarray(
        rslab.reshape(NB, NCHUNK // RTB, RTB, 128, RN).transpose(0, 1, 3, 2, 4)
    ).reshape(NB, NCHUNK // RTB, 128, RTB * RN)
    # compact main rows: row blk*8+k only has cols [(k)*16, (k+1)*16) nonzero
    rowc = np.zeros((128, 128, 16), dtype=np.float32)
    for k in range(8):
        rowc[:, k::8, :] = rowlhs[:, 0:128, :][:, k::8, k * 16 : (k + 1) * 16]
    return {
        "rlhs": rlhs_t.astype(ml_dtypes.bfloat16),
        "rslab": rslab_t,
        "idx": idx,
        "rowc": rowc.astype(ml_dtypes.bfloat16).reshape(128, 128 * 16),
        "rowlhs": rowlhs[:, 128:, :].astype(ml_dtypes.bfloat16).reshape(
            128, 64 * 128
        ),
    }


def kernel(xyz, n_photons, coeffs, inv_voxel_size, psf_center, img_size):
    global _compiled
    xyz = np.asarray(xyz, dtype=np.float32)
    n_photons = np.asarray(n_photons, dtype=np.float32)
    coeffs = np.asarray(coeffs, dtype=np.float32)
    inv_voxel_size = np.asarray(inv_voxel_size, dtype=np.float32)
    psf_center = np.asarray(psf_center, dtype=np.float32)

    in_maps = _host_prep(xyz, n_photons, coeffs, inv_voxel_size, psf_center)

    if _compiled is None:
        _compiled = _build_bass()
    nc = _compiled

    res = run_bass_kernel_spmd(nc, in_maps, core_ids=list(range(B)))
    outs = []
    for b in range(B):
        c4 = res.results[b]["out"].astype(np.float32).reshape(8, 16, NBLK, XW)
        outs.append(c4.transpose(1, 2, 0, 3).reshape(F, IMG, IMG))
    return np.stack(outs, axis=0)


# revision 59
# speedup vs baseline: 1.0649x; 1.0038x over previous
import sys

sys.path.insert(0, "/opt/trn_rl_repo")

import numpy as np
import ml_dtypes

import concourse.bass as bass
import concourse.tile as tile
from concourse import bacc, mybir
from concourse.bass_utils import run_bass_kernel_spmd

# ---- problem constants (hardcoded per contract) ----
B, N, F = 8, 512, 16
D, PP = 150, 26
IMG = 128
NB = 4                  # render batches of 128 z-sorted emitters
NCHUNK = 16             # 128-row K chunks per batch (2 z-slabs of 64);
                        # bin-packed batches have <=30 distinct z each
RTB = 8                 # rt chunks per DMA
RN = PP * PP            # 676
W184 = 160              # dram row stride (elements); zero gap 134 >= 113 max backread
LEAD = 8                # leading zero rows per partition
TAIL = 7                # trailing zero rows
JROWS = LEAD + NB * PP + TAIL   # 119
SEG = JROWS * W184      # 21896 elements per partition
NBLK = 16
NSP = 4                 # spill cells, each serving 4 blocks
QI = NBLK + NSP         # 16 main octs + 4 spill quads = 20 gather chunks
CHO = 7 * W184 + 128    # 1416 elements per oct chunk (8 rows)
CHS = 3 * W184 + 128    # 680 elements per spill chunk (4 rows)
XW = 128                # canvas x window [26,154)

_compiled = None


def _build_bass(debug=False):
    nc = bacc.Bacc()
    f32 = mybir.dt.float32
    bf16 = mybir.dt.bfloat16
    i32 = mybir.dt.int32

    rlhs_d = nc.declare_dram_parameter(
        "rlhs", [NB, 128, NCHUNK * 128], bf16, isOutput=False
    )
    rslab_d = nc.declare_dram_parameter(
        "rslab", [NB, NCHUNK // RTB, 128, RTB * RN], bf16, isOutput=False
    )
    idx_d = nc.declare_dram_parameter("idx", [128, QI], i32, isOutput=False)
    # main oct rows compact (cols (row%8)*16..+16 only); spill rows full width
    rowc_d = nc.declare_dram_parameter("rowc", [128, 128 * 16], bf16, isOutput=False)
    rowlhs_d = nc.declare_dram_parameter(
        "rowlhs", [128, 64 * 128], bf16, isOutput=False
    )
    out_d = nc.declare_dram_parameter("out", [128, NBLK * XW], bf16, isOutput=True)
    dbg = {}
    if debug:
        dbg["rend"] = nc.declare_dram_parameter(
            "dbg_rend", [128, SEG], bf16, isOutput=True
        )

    with tile.TileContext(nc) as tc:
        with (
            tc.tile_pool(name="big", bufs=1) as big_pool,
            tc.tile_pool(name="lt", bufs=2) as lt_pool,
            tc.tile_pool(name="rt", bufs=5) as rt_pool,
            tc.tile_pool(name="psr", bufs=2, space="PSUM") as psr_pool,
            tc.tile_pool(name="psc", bufs=4, space="PSUM") as psc_pool,
            tc.tile_pool(name="dram", bufs=1, space="DRAM") as dram_pool,
        ):
            idx_t = big_pool.tile([128, QI], i32, tag="idx")

            rend = big_pool.tile([128, SEG], bf16, tag="rend")
            nc.vector.memset(rend[:].bitcast(i32), 0)

            rows_d = dram_pool.tile([128, SEG], bf16, tag="rows")
            rowlhs_t = big_pool.tile([128, 192 * 128], bf16, tag="rowlhs")
            rowc_t = big_pool.tile([128, 128 * 16], bf16, tag="rowc")
            # zero the main-row region; DVE copies later fill the 16-col blocks
            nc.vector.memset(rowlhs_t[:, : 128 * 128].bitcast(i32), 0)

            # zero lead + tail rows depend only on the memset; dump them early.
            # dumps ride the (otherwise idle) gpsimd SWDGE ring so the sync
            # ring stays clear for the rt slab stream.
            nc.gpsimd.dma_start(rows_d[:, 0 : LEAD * W184], rend[:, 0 : LEAD * W184])
            nc.gpsimd.dma_start(
                rows_d[:, (JROWS - TAIL) * W184 :], rend[:, (JROWS - TAIL) * W184 :]
            )

            for m in range(NB):
                lt = lt_pool.tile([128, NCHUNK * 128], bf16, tag="lt")
                nc.sync.dma_start(lt[:], rlhs_d[m])
                ps = psr_pool.tile([128, RN], f32, tag="ps")
                for cb in range(NCHUNK // RTB):
                    rt = rt_pool.tile([128, RTB * RN], bf16, tag="rt")
                    (nc.scalar if cb % 2 == 0 else nc.sync).dma_start(
                        rt[:], rslab_d[m, cb]
                    )
                    for cc in range(RTB):
                        c = cb * RTB + cc
                        for n0, n1 in ((0, 512), (512, RN)):
                            nc.tensor.matmul(
                                ps[:, n0:n1],
                                lhsT=lt[:, c * 128 : (c + 1) * 128],
                                rhs=rt[:, cc * RN + n0 : cc * RN + n1],
                                start=(c == 0),
                                stop=(c == NCHUNK - 1),
                            )
                if m == 2:
                    # idx only needed by the gathers; load out of startup path
                    nc.scalar.dma_start(idx_t[:], idx_d[:])
                if m == 3:
                    # row-MM tables load late (render bandwidth) but small
                    # enough (2.9MB) to complete alongside the last dump
                    nc.scalar.dma_start(rowc_t[:], rowc_d[:])
                    nc.scalar.dma_start(
                        rowlhs_t[:, 128 * 128 :], rowlhs_d[:]
                    )
                    # expand compact main rows on the (idle) DVE:
                    # row blk*8+k only has cols [k*16, k*16+16) nonzero
                    rl4 = (
                        rowlhs_t[:]
                        .rearrange("p (r c) -> p r c", r=192)[:, 0:128, :]
                        .rearrange("p (b k) c -> p b k c", k=8)
                    )
                    rc4 = rowc_t[:].rearrange("p (b k f) -> p b k f", k=8, f=16)
                    for k in range(8):
                        nc.vector.tensor_copy(
                            rl4[:, :, k, k * 16 : (k + 1) * 16], rc4[:, :, k, :]
                        )
                # copy+dump in halves so the final dump completes sooner; the
                # copies run on the (render-idle) DVE so the scalar sequencer
                # stays free for its slab DMA issues
                for r0, r1 in ((0, 13), (13, 26)):
                    nc.vector.tensor_copy(
                        rend[:].rearrange("p (j w) -> p j w", j=JROWS)[
                            :, m * 26 + LEAD + r0 : m * 26 + LEAD + r1, 0:26
                        ],
                        ps[:].rearrange("p (h w) -> p h w", h=26)[:, r0:r1, :],
                    )
                    # batch 3's halves go to the two HWDGE rings: transfers run
                    # in parallel and the gpsimd ring is free for the gathers
                    # queued immediately behind
                    eng = (
                        (nc.sync if r0 == 0 else nc.scalar)
                        if m == 3
                        else nc.gpsimd
                    )
                    eng.dma_start(
                        rows_d[
                            :,
                            (m * 26 + LEAD + r0) * W184 : (m * 26 + LEAD + r1) * W184,
                        ],
                        rend[
                            :,
                            (m * 26 + LEAD + r0) * W184 : (m * 26 + LEAD + r1) * W184,
                        ],
                    )
            strip_o = big_pool.tile([128, NBLK * CHO], bf16, tag="strip_o")
            strip_o3 = strip_o[:].rearrange("p (q x) -> p q x", q=NBLK)
            strip_s = big_pool.tile([128, NSP * CHS], bf16, tag="strip_s")
            strip_s3 = strip_s[:].rearrange("p (q x) -> p q x", q=NSP)
            rows_flat = rows_d[:].rearrange("p (n o) -> (p n) o", o=1)
            # first oct, then the group's spill cell, then remaining octs:
            # each block's main row-MMs start at its own oct, spill data is
            # already there, and the tail waits only on the last single oct
            for j in range(NSP):
                for pos, col in enumerate(
                    [4 * j, NBLK + j, 4 * j + 1, 4 * j + 2, 4 * j + 3]
                ):
                    out_ap = (
                        strip_s3[:, j, :] if pos == 1 else strip_o3[:, col, :]
                    )
                    nc.gpsimd.indirect_dma_start(
                        out=out_ap,
                        out_offset=None,
                        in_=rows_flat,
                        in_offset=bass.IndirectOffsetOnAxis(
                            ap=idx_t[:, col : col + 1], axis=0
                        ),
                    )

            if debug:
                nc.sync.dma_start(dbg["rend"][:], rend[:])

            canvas = big_pool.tile([128, NBLK * XW], bf16, tag="canvas")
            rowlhs3 = rowlhs_t[:].rearrange("p (y m) -> p y m", y=192)
            for blk in range(NBLK):
                pc = psc_pool.tile([128, XW], f32, tag="pc")
                for k in range(8):
                    nc.tensor.matmul(
                        pc[:],
                        lhsT=rowlhs3[:, blk * 8 + k, :],
                        rhs=strip_o3[:, blk, k * W184 : k * W184 + XW],
                        start=(k == 0),
                        stop=False,
                    )
                for k in range(4):
                    nc.tensor.matmul(
                        pc[:],
                        lhsT=rowlhs3[:, 128 + blk * 4 + k, :],
                        rhs=strip_s3[:, blk // 4, k * W184 : k * W184 + XW],
                        start=False,
                        stop=(k == 3),
                    )
                nc.scalar.copy(
                    out=canvas[:, blk * XW : (blk + 1) * XW], in_=pc[:]
                )
                # ship each block as soon as it drains (tail shrinks to the
                # last block's small DMA)
                (nc.sync if blk % 2 == 0 else nc.scalar).dma_start(
                    out_d[:, blk * XW : (blk + 1) * XW],
                    canvas[:, blk * XW : (blk + 1) * XW],
                )
    if not nc.is_finalized():
        nc.finalize()
    return nc


def _host_prep(xyz, n_photons, coeffs, inv_voxel_size, psf_center):
    u = xyz * inv_voxel_size
    u = u.copy()
    u[..., :2] -= psf_center[:2]
    u[..., 2] += psf_center[2]
    u_floor = np.floor(u)
    frac = u - u_floor
    ui = u_floor.astype(np.int32)
    x_idx = ui[..., 0] + PP
    y_idx = ui[..., 1] + PP
    z_idx = ui[..., 2]
    frac[..., :2] = 1.0 - frac[..., :2]

    p4 = frac[..., None] ** np.arange(4, dtype=np.float32)
    vx, vy, vz = p4[..., 0, :], p4[..., 1, :], p4[..., 2, :]
    series = (
        vz[..., :, None, None] * vx[..., None, :, None] * vy[..., None, None, :]
    ).reshape(B, N, 64)

    slab = np.ascontiguousarray(coeffs.transpose(0, 3, 1, 2).reshape(D, 64, RN))

    in_maps = []
    for b in range(B):
        in_maps.append(
            _prep_one(x_idx[b], y_idx[b], z_idx[b], series[b], n_photons[b], slab)
        )
    return in_maps


def _prep_one(x_idx, y_idx, z_idx, series, photons, slab):
    # bin-pack z-groups into NB batches of exactly 128 emitters, balancing
    # distinct-z per batch (<= 2*NCHUNK); batches need not be z-contiguous
    zs = np.unique(z_idx)
    groups = [np.where(z_idx == z)[0] for z in zs]
    groups.sort(key=len, reverse=True)
    counts = [0] * NB
    nz = [0] * NB
    bins = [[] for _ in range(NB)]
    for g in groups:
        take = list(g)
        while take:
            cand = [i for i in range(NB) if counts[i] < 128]
            i = min(cand, key=lambda i: (counts[i], nz[i]))
            space = 128 - counts[i]
            put = take[:space]
            bins[i].extend(put)
            counts[i] += len(put)
            nz[i] += 1
            take = take[space:]
    assert all(c == 128 for c in counts)
    assert max(nz) <= 2 * NCHUNK, nz
    order = np.array([e for b in bins for e in b], dtype=np.int64)
    pos = np.empty(N, dtype=np.int64)
    pos[order] = np.arange(N)

    rlhs = np.zeros((NB, NCHUNK, 128, 128), dtype=np.float32)
    rslab = np.zeros((NB, NCHUNK, 128, RN), dtype=ml_dtypes.bfloat16)
    for m in range(NB):
        es = order[m * 128 : (m + 1) * 128]
        zlist = np.unique(z_idx[es])
        assert len(zlist) <= 2 * NCHUNK
        zpos = {z: i for i, z in enumerate(zlist)}
        for i, z in enumerate(zlist):
            rslab[m, i // 2, 64 * (i % 2) : 64 * (i % 2) + 64, :] = slab[z]
        for col, e in enumerate(es):
            zi = zpos[z_idx[e]]
            rlhs[m, zi // 2, 64 * (zi % 2) : 64 * (zi % 2) + 64, col] = series[e]

    # per-block packing: main cells are 8-row octs (idx col = blk, lhsT rows
    # blk*8+k), overflow contributors split into 4-row spill windows (idx col
    # = NBLK+blk, lhsT rows 128 + blk*4+k); junk rows get zero photon weights
    idx = np.full((128, QI), 26, dtype=np.int32)
    rowlhs = np.zeros((128, 192, 128), dtype=np.float32)
    fill = np.zeros(QI, dtype=np.int64)

    def emit(cell, y0, nrows, lrow0, e):
        """emitter e's piece with an nrows-row window starting at canvas-crop
        row y0, placed in gather column `cell`, lhsT rows lrow0+k"""
        s = fill[cell]
        assert s < 128, f"cell overflow {cell}"
        fill[cell] += 1
        r0 = y0 + 26 - y_idx[e]  # patch row at window row 0 (may be <0)
        p = pos[e] % 128
        jb = (pos[e] // 128) * 26 + LEAD
        idx[s, cell] = p * SEG + (jb + r0) * W184 + 26 - x_idx[e]
        for k in range(nrows):
            Y = y0 + k
            r = r0 + k
            if 0 <= r < 26 and 0 <= Y < IMG:
                col0 = (Y % 8) * 16
                rowlhs[s, lrow0 + k, col0 : col0 + 16] = photons[e]

    for blk in range(NBLK):
        es = np.where((y_idx > 8 * blk) & (y_idx <= 8 * blk + 33))[0]
        for i, e in enumerate(es):
            if i < 128:
                emit(blk, 8 * blk, 8, blk * 8, e)
            else:
                for h in range(2):
                    y0 = 8 * blk + 4 * h
                    if (y_idx[e] > y0) and (y_idx[e] <= y0 + 29):
                        emit(NBLK + blk // 4, y0, 4, 128 + blk * 4, e)

    # partition-contiguous DRAM layouts (128 descriptors per DMA)
    rlhs_t = np.ascontiguousarray(rlhs.transpose(0, 2, 1, 3)).reshape(
        NB, 128, NCHUNK * 128
    )
    rslab_t = np.ascontiguousarray(
        rslab.reshape(NB, NCHUNK // RTB, RTB, 128, RN).transpose(0, 1, 3, 2, 4)
    ).reshape(NB, NCHUNK // RTB, 128, RTB * RN)
    # compact main rows: row blk*8+k only has cols [(k)*16, (k+1)*16) nonzero
    rowc = np.zeros((128, 128, 16), dtype=np.float32)
    for k in range(8):
        rowc[:, k::8, :] = rowlhs[:, 0:128, :][:, k::8, k * 16 : (k + 1) * 16]
    return {
        "rlhs": rlhs_t.astype(ml_dtypes.bfloat16),
        "rslab": rslab_t,
        "idx": idx,
        "rowc": rowc.astype(ml_dtypes.bfloat16).reshape(128, 128 * 16),
        "rowlhs": rowlhs[:, 128:, :].astype(ml_dtypes.bfloat16).reshape(
            128, 64 * 128
        ),
    }


def kernel(xyz, n_photons, coeffs, inv_voxel_size, psf_center, img_size):
    global _compiled
    xyz = np.asarray(xyz, dtype=np.float32)
    n_photons = np.asarray(n_photons, dtype=np.float32)
    coeffs = np.asarray(coeffs, dtype=np.float32)
    inv_voxel_size = np.asarray(inv_voxel_size, dtype=np.float32)
    psf_center = np.asarray(psf_center, dtype=np.float32)

    in_maps = _host_prep(xyz, n_photons, coeffs, inv_voxel_size, psf_center)

    if _compiled is None:
        _compiled = _build_bass()
    nc = _compiled

    res = run_bass_kernel_spmd(nc, in_maps, core_ids=list(range(B)))
    outs = []
    for b in range(B):
        c4 = res.results[b]["out"].astype(np.float32).reshape(8, 16, NBLK, XW)
        outs.append(c4.transpose(1, 2, 0, 3).reshape(F, IMG, IMG))
    return np.stack(outs, axis=0)
